# revision 1
# baseline (speedup 1.0000x reference)
"""LoRA multi-head attention kernel for 8 Trainium2 NeuronCores.

Problem: q = x_q@(Wq.T + Aq@Bq*2) + bq ; k = x_k@Wk.T + bk ;
         v = x_v@(Wv.T + Av@Bv*2) + bv ; MHA over 16 heads, D=64,
         out = attn_out @ Wo.T + bo.   Shapes: x [2048, 4, 1024].

Sharding: core c handles batch b = c//2 and head-group hg = c%2
(8 heads = 512 channels). LoRA weights are merged on the host
(mathematically exact), the 1/sqrt(D) score scale is folded into Wk/bk,
and x is transposed on the host so every matmul contracts over the
partition dimension. Each core computes a partial output
(its 512 channels through Wo); the host sums the two partials per batch.

Device layout per core:
  qT/kT  [ch, tok] ; v [tok, ch] augmented with a ones column so the
  attn@v matmul also produces the softmax denominator (scores are
  exponentiated WITHOUT max subtraction -- safe here, |scores| < ~6 --
  and normalization happens after attn@v on the [D, S] output, 32x
  cheaper than normalizing the attention matrix).
All matmuls run as float32r (full PE rate at free dim 512).
"""

import sys

import numpy as np

sys.path.insert(0, "/opt/trn_rl_repo")

from contextlib import ExitStack  # noqa: E402

import concourse.bass as bass  # noqa: E402
import concourse.tile as tile  # noqa: E402
from concourse import bacc, mybir  # noqa: E402
from concourse.bass_utils import run_bass_kernel_spmd  # noqa: E402

F32 = mybir.dt.float32
F32R = mybir.dt.float32r
AF = mybir.ActivationFunctionType
ALU = mybir.AluOpType

E = 1024
D = 64
NHC = 8            # heads per core
CH = NHC * D       # 512 output channels per core
KT = E // 128      # k-tiles over the E contraction
NCORES = 8
B = 4


def build_program(S=2048, num_devices=8):
    TB = 256 if S >= 512 else S     # token block for projections
    NTB = S // TB
    NSB = S // 512 if S >= 512 else 1
    SBK = S // NSB                  # s-block width
    NTT = S // 128                  # t tiles
    MT = S // 128                   # tok tiles (v projection / output)
    NM = CH // 128                  # ch tiles per core (4)

    nc = bacc.Bacc(
        "TRN2", target_bir_lowering=False, debug=False, num_devices=num_devices
    )

    def dram(name, shape, out=False, dt=F32):
        kind = "ExternalOutput" if out else "ExternalInput"
        return nc.dram_tensor(name, shape, dt, kind=kind).ap()

    xq = dram("xq", [128, KT, S], dt=F32R)
    xk = dram("xk", [128, KT, S], dt=F32R)
    xv = dram("xv", [128, KT, S], dt=F32R)
    wq = dram("wq", [128, KT, CH], dt=F32R)
    wk = dram("wk", [128, KT, CH], dt=F32R)
    wv = dram("wv", [128, KT, CH], dt=F32R)
    wo = dram("wo", [128, NM, E // 512, 512], dt=F32R)
    bq = dram("bq", [128, NM])
    bk = dram("bk", [128, NM])
    bv = dram("bv", [128, CH])
    bo = dram("bo", [128, E])
    onesd = dram("onesd", [64], dt=F32R)
    out = dram("out", [S, E], out=True)

    with tile.TileContext(nc) as tc, ExitStack() as top:
        persist = top.enter_context(tc.tile_pool(name="persist", bufs=1))
        qT = persist.tile([128, NM, S], F32R)          # [ch%128, ch//128, tok]
        kT = persist.tile([128, NM, S], F32R)
        vaug = persist.tile([128, NTT, NHC, D + 1], F32R)  # [tok%128, ttile, h, d+1]
        aoT = persist.tile([128, NM, S], F32R)         # attention out, [ch, tok]
        bq_sb = persist.tile([128, NM], F32)
        bk_sb = persist.tile([128, NM], F32)
        bv_sb = persist.tile([128, CH], F32)
        ones_sb = persist.tile([1, D], F32R)
        nc.sync.dma_start(out=bq_sb, in_=bq)
        nc.sync.dma_start(out=bk_sb, in_=bk)
        nc.sync.dma_start(out=bv_sb, in_=bv)
        nc.gpsimd.dma_start(out=ones_sb, in_=onesd[None, :])
        nc.vector.memset(vaug[:, :, :, D:D + 1].bitcast(F32), 1.0)

        # ---------------- Phase A: q/k/v projections ----------------
        with tc.tile_pool(name="wts", bufs=1) as wpool, \
             tc.tile_pool(name="xs", bufs=3) as xpool, \
             tc.tile_pool(name="pps", bufs=3, space="PSUM") as ppool:
            wq_sb = wpool.tile([128, KT, CH], F32R, tag="wq")
            wk_sb = wpool.tile([128, KT, CH], F32R, tag="wk")
            wv_sb = wpool.tile([128, KT, CH], F32R, tag="wv")
            nc.sync.dma_start(out=wq_sb, in_=wq)
            nc.sync.dma_start(out=wk_sb, in_=wk)
            nc.sync.dma_start(out=wv_sb, in_=wv)

            # k then q: qT/kT[ch, tok] = W.T @ x.T  (+ bias per partition)
            for xap, w_sb, b_sb, dst in (
                (xk, wk_sb, bk_sb, kT),
                (xq, wq_sb, bq_sb, qT),
            ):
                for nb in range(NTB):
                    xt = xpool.tile([128, KT, TB], F32R, tag="x")
                    nc.sync.dma_start(out=xt, in_=xap[:, :, nb * TB:(nb + 1) * TB])
                    for m in range(NM):
                        ps = ppool.tile([128, TB], F32, tag="pp")
                        for k in range(KT):
                            nc.tensor.matmul(
                                ps,
                                (w_sb[:, k, m * 128:(m + 1) * 128]),
                                (xt[:, k, :]),
                                start=(k == 0),
                                stop=(k == KT - 1),
                            )
                        nc.vector.tensor_scalar(
                            out=dst[:, m, nb * TB:(nb + 1) * TB],
                            in0=ps,
                            scalar1=b_sb[:, m:m + 1],
                            scalar2=None,
                            op0=ALU.add,
                        )
            # v: v[tok, ch] = x @ Wv_eff  (+ bias along free dim)
            for nb in range(NTB):
                xt = xpool.tile([128, KT, TB], F32R, tag="x")
                nc.sync.dma_start(out=xt, in_=xv[:, :, nb * TB:(nb + 1) * TB])
                for mi in range(TB // 128):
                    mt = nb * (TB // 128) + mi
                    ps = ppool.tile([128, CH], F32, tag="pp")
                    for k in range(KT):
                        nc.tensor.matmul(
                            ps,
                            (xt[:, k, mi * 128:(mi + 1) * 128]),
                            (wv_sb[:, k, :]),
                            start=(k == 0),
                            stop=(k == KT - 1),
                        )
                    nc.vector.tensor_add(
                        out=vaug[:, mt, :, 0:D],
                        in0=ps.rearrange("p (h d) -> p h d", d=D),
                        in1=bv_sb.rearrange("p (h d) -> p h d", d=D),
                    )

        # ---------------- Phase B: attention ----------------
        # scores_T[t, s] = k_scaled @ q.T per head; exp; oaug = [v | 1].T @ exp
        # (row D of oaug = softmax denominator); normalize into aoT.
        with tc.tile_pool(name="scps", bufs=1, space="PSUM") as scpool, \
             tc.tile_pool(name="oaps", bufs=1, space="PSUM") as opool, \
             tc.tile_pool(name="bcps", bufs=1, space="PSUM") as bcpool, \
             tc.tile_pool(name="exs", bufs=4) as expool, \
             tc.tile_pool(name="nrm", bufs=3) as npool:
            for hp in range(NM):
                for sb_i in range(NSB):
                    ssl = slice(sb_i * SBK, (sb_i + 1) * SBK)
                    oaugs = [
                        opool.tile(
                            [D + 1, SBK], F32, tag=f"oaug{h_in}", name=f"oaug{h_in}"
                        )
                        for h_in in range(2)
                    ]
                    for tt2 in range(NTT // 2):
                        for h_in in range(2):
                            h = 2 * hp + h_in
                            p0 = h_in * 64
                            sc = scpool.tile([128, 2, SBK], F32, tag=f"sc{h_in}")
                            for j in range(2):
                                tt = tt2 * 2 + j
                                nc.tensor.matmul(
                                    sc[:, j, :],
                                    (kT[p0:p0 + 64, hp, tt * 128:(tt + 1) * 128]),
                                    (qT[p0:p0 + 64, hp, ssl]),
                                    start=True,
                                    stop=True,
                                )
                            ex = expool.tile([128, 2, SBK], F32R, tag=f"ex{h_in}")
                            nc.scalar.activation(out=ex, in_=sc, func=AF.Exp)
                            for j in range(2):
                                tt = tt2 * 2 + j
                                nc.tensor.matmul(
                                    oaugs[h_in],
                                    (vaug[:, tt, h, :]),
                                    (ex[:, j, :]),
                                    start=(tt == 0),
                                    stop=(tt == NTT - 1),
                                )
                    for h_in in range(2):
                        p0 = h_in * 64
                        recip32 = npool.tile([1, SBK], F32, tag="recip32")
                        nc.vector.reciprocal(out=recip32, in_=oaugs[h_in][D:D + 1, :])
                        recip = npool.tile([1, SBK], F32R, tag="recip")
                        nc.vector.tensor_copy(out=recip, in_=recip32)
                        bc = bcpool.tile([D, SBK], F32, tag="bc")
                        nc.tensor.matmul(
                            bc, (ones_sb), (recip), start=True, stop=True
                        )
                        rb = npool.tile([D, SBK], F32, tag="rb")
                        nc.vector.tensor_copy(out=rb, in_=bc)
                        nc.vector.tensor_mul(
                            out=aoT[p0:p0 + 64, hp, ssl],
                            in0=oaugs[h_in][0:D, :],
                            in1=rb,
                        )

        # ---------------- Phase C: output projection (partial Wo) ----------------
        with tc.tile_pool(name="wos", bufs=1) as wopool, \
             tc.tile_pool(name="wops", bufs=2, space="PSUM") as wpp, \
             tc.tile_pool(name="outs", bufs=3) as outpool:
            wo_sb = wopool.tile([128, NM, E // 512, 512], F32R)
            bo_sb = wopool.tile([128, E], F32)
            nc.sync.dma_start(out=wo_sb, in_=wo)
            nc.sync.dma_start(out=bo_sb, in_=bo)
            for mt in range(MT):
                for nb2 in range(E // 512):
                    ps = wpp.tile([128, 512], F32, tag="wops")
                    for kc in range(NM):
                        nc.tensor.matmul(
                            ps,
                            (aoT[:, kc, mt * 128:(mt + 1) * 128]),
                            (wo_sb[:, kc, nb2, :]),
                            start=(kc == 0),
                            stop=(kc == NM - 1),
                        )
                    ot = outpool.tile([128, 512], F32, tag="ot")
                    nc.vector.tensor_add(
                        out=ot, in0=ps, in1=bo_sb[:, nb2 * 512:(nb2 + 1) * 512]
                    )
                    nc.sync.dma_start(
                        out=out[mt * 128:(mt + 1) * 128, nb2 * 512:(nb2 + 1) * 512],
                        in_=ot,
                    )

    nc.compile()
    return nc


_PROG = {}


def _get_prog(S=2048, num_devices=8):
    key = (S, num_devices)
    if key not in _PROG:
        _PROG[key] = build_program(S, num_devices)
    return _PROG[key]


def _tile_x(x2d):
    # [S, E] slice -> [128, KT, S] with element (p, k, t) = x2d[t, k*128+p]
    S = x2d.shape[0]
    xt = np.ascontiguousarray(x2d.T.astype(np.float32))
    return np.ascontiguousarray(xt.reshape(KT, 128, S).transpose(1, 0, 2))


def _tile_w(weff, ch0):
    w = weff[:, ch0:ch0 + CH]
    return np.ascontiguousarray(
        w.reshape(KT, 128, CH).transpose(1, 0, 2).astype(np.float32)
    )


def prep_in_maps(x_q, x_k, x_v, Wq, bq, Aq, Bq, Wk, bk, Wv, bv, Av, Bv, Wo, bo):
    x_q = np.asarray(x_q, np.float32)
    x_k = np.asarray(x_k, np.float32)
    x_v = np.asarray(x_v, np.float32)
    scaling = 2.0  # lora_alpha / r = 32 / 16
    wq_eff = (np.asarray(Wq).T + (np.asarray(Aq) @ np.asarray(Bq)) * scaling).astype(
        np.float32
    )
    wv_eff = (np.asarray(Wv).T + (np.asarray(Av) @ np.asarray(Bv)) * scaling).astype(
        np.float32
    )
    wk_s = (np.asarray(Wk).T / 8.0).astype(np.float32)  # sqrt(D) folded in
    bk_s = (np.asarray(bk) / 8.0).astype(np.float32)
    bq = np.asarray(bq, np.float32)
    bv = np.asarray(bv, np.float32)
    bo = np.asarray(bo, np.float32)
    woT = np.ascontiguousarray(np.asarray(Wo).T.astype(np.float32))

    nbatch = x_q.shape[1]
    in_maps = []
    for c in range(2 * nbatch):
        b = c // 2
        hg = c % 2
        ch0 = hg * CH
        wo_c = np.ascontiguousarray(
            woT[ch0:ch0 + CH, :].reshape(CH // 128, 128, E // 512, 512)
            .transpose(1, 0, 2, 3)
        )
        in_maps.append({
            "xq": _tile_x(x_q[:, b, :]),
            "xk": _tile_x(x_k[:, b, :]),
            "xv": _tile_x(x_v[:, b, :]),
            "wq": _tile_w(wq_eff, ch0),
            "wk": _tile_w(wk_s, ch0),
            "wv": _tile_w(wv_eff, ch0),
            "wo": wo_c,
            "bq": np.ascontiguousarray(bq[ch0:ch0 + CH].reshape(CH // 128, 128).T),
            "bk": np.ascontiguousarray(bk_s[ch0:ch0 + CH].reshape(CH // 128, 128).T),
            "bv": np.ascontiguousarray(np.broadcast_to(bv[ch0:ch0 + CH], (128, CH))),
            "onesd": np.ones(64, np.float32),
            "bo": (
                np.ascontiguousarray(np.broadcast_to(bo, (128, E)))
                if hg == 0
                else np.zeros((128, E), np.float32)
            ),
        })
    return in_maps


def gather_out(results, nbatch):
    return np.stack(
        [results[2 * b]["out"] + results[2 * b + 1]["out"] for b in range(nbatch)],
        axis=1,
    )


def kernel(**inputs):
    nc = _get_prog(2048, 8)
    in_maps = prep_in_maps(**inputs)
    res = run_bass_kernel_spmd(nc, in_maps, core_ids=list(range(NCORES)))
    return gather_out(res.results, B)



# revision 75
# speedup vs baseline: 1.3171x; 1.3171x over previous
"""LoRA multi-head attention kernel for 8 Trainium2 NeuronCores.

Problem: q = x_q@(Wq.T + Aq@Bq*2) + bq ; k = x_k@Wk.T + bk ;
         v = x_v@(Wv.T + Av@Bv*2) + bv ; MHA over 16 heads, D=64,
         out = attn_out @ Wo.T + bo.   Shapes: x [2048, 4, 1024].

Sharding: core c handles batch b = c//2 and head-group hg = c%2
(8 heads = 512 output channels), processed as 4 head-PAIRS so every
projection matmul uses the full 128 PE output partitions.

Math simplifications (exact):
  * LoRA weights merged on host; 1/sqrt(D) folded into Wk.
  * bk dropped entirely: q.(k+bk) adds a per-query constant to every
    score, which softmax cancels.
  * bv dropped on-device: sum(attn)=1, so + bv@Wo.T is a constant
    row added on the host. bo is also added on the host.
  * Per-pair partial output projections are summed on the host
    (together with the partner core's partials).

Per pair the attention runs "orientation B": scores land as
sc[t, s] tiles, exp'd by the Act engine into bf16, and attn@v uses
ex[t, s-tile] as the *stationary* operand with [v | 1] as the moving
operand (free dim 65).  That halves the tensor-engine rows of attn@v
versus the [d+1, s]-output orientation, and the softmax denominator
(column 64 of the output) becomes a per-partition scalar, so
normalization is a cheap DVE tensor_scalar.  The normalized [s, c]
output is PE-transposed to [c, s] to feed the Wo projection.

The emission interleaves the NEXT pair's projections and the PREVIOUS
pair's Wo-projection tiles between attention iterations so the tensor
engine never idles while the Act engine grinds the exps (the Act
engine's 262k exp-elements/core at 0.83 ns/elem is nearly as expensive
as all matmuls).

Dtypes: x/W inputs bf16 (halves DMA), qT/kT kept f32r, exp output and
the v/Wo path bf16.
"""

import sys

import numpy as np

sys.path.insert(0, "/opt/trn_rl_repo")

from contextlib import ExitStack  # noqa: E402

import ml_dtypes  # noqa: E402

import concourse.bass as bass  # noqa: E402
import concourse.tile as tile  # noqa: E402
from concourse import bacc, mybir  # noqa: E402
from concourse.bass_utils import run_bass_kernel_spmd  # noqa: E402

F32 = mybir.dt.float32
F32R = mybir.dt.float32r
BF16 = mybir.dt.bfloat16
AF = mybir.ActivationFunctionType
ALU = mybir.AluOpType

E = 1024
D = 64
CH = 512           # output channels per core
NPAIR = 4          # head pairs per core
KT = E // 128      # 8 k-tiles over the E contraction
NCORES = 8
B = 4
BF_NP = ml_dtypes.bfloat16


def build_program(S=2048, num_devices=8):
    NTT = S // 128          # 16 token tiles
    NSB = S // 512          # 4 query blocks
    NCK = S // 512          # 4 projection token chunks
    CKW = 512

    nc = bacc.Bacc(
        "TRN2", target_bir_lowering=False, debug=False, num_devices=num_devices
    )

    def dram(name, shape, dt=F32, out=False):
        kind = "ExternalOutput" if out else "ExternalInput"
        return nc.dram_tensor(name, shape, dt, kind=kind).ap()

    NHC = S // 256          # 8 half-chunks per x tensor
    # x: half-chunk-major so each DMA writes (and each reader reads) one
    # contiguous block -- the tile framework's range-based subtile deps
    # would otherwise serialize readers on ALL of a tensor's chunk DMAs
    xq = dram("xq", [128, NHC, KT, 256], BF16)
    xk = dram("xk", [128, NHC, KT, 256], BF16)
    xv = dram("xv", [128, NHC, KT, 256], BF16)
    # weights: pair-major for the same reason
    wq = dram("wq", [128, NPAIR, KT, 128], BF16)
    wk = dram("wk", [128, NPAIR, KT, 128], BF16)
    wv = dram("wv", [128, NPAIR, KT, 128], BF16)
    wo = dram("wo", [128, NPAIR, E], BF16)
    bq = dram("bq", [128, NPAIR])
    idm = dram("idm", [128, 128], BF16)
    out = dram("out", [NPAIR // 2, S, E], out=True)

    with tile.TileContext(nc) as tc, ExitStack() as top:
        persist = top.enter_context(tc.tile_pool(name="persist", bufs=1))
        ring = top.enter_context(tc.tile_pool(name="ring", bufs=2))
        expool = top.enter_context(tc.tile_pool(name="expool", bufs=14))
        npool = top.enter_context(tc.tile_pool(name="npool", bufs=8))
        otpool = top.enter_context(tc.tile_pool(name="otpool", bufs=4))
        pp = top.enter_context(tc.tile_pool(name="pp", bufs=2, space="PSUM"))
        scp = top.enter_context(tc.tile_pool(name="scp", bufs=2, space="PSUM"))
        opp = top.enter_context(tc.tile_pool(name="opp", bufs=2, space="PSUM"))

        xq_sb = persist.tile([128, NHC, KT, 256], BF16)
        xk_sb = persist.tile([128, NHC, KT, 256], BF16)
        xv_sb = persist.tile([128, NHC, KT, 256], BF16)
        wq_sb = persist.tile([128, NPAIR, KT, 128], BF16)
        wk_sb = persist.tile([128, NPAIR, KT, 128], BF16)
        wv_sb = persist.tile([128, NPAIR, KT, 128], BF16)
        wo_sb = persist.tile([128, NPAIR, E], BF16)
        bq_sb = persist.tile([128, NPAIR], F32)
        idm_sb = persist.tile([128, 128], BF16)

        # DMA order tuned for startup: the first scores only need pair-0's
        # wk/wq column slices, a half xk chunk and the first xq chunk
        # (~1.8 MB); everything else streams in behind while attention is
        # already running (the sim's engines let ready instructions bypass
        # ones parked on late DMAs).
        nc.sync.dma_start(out=wk_sb[:, 0], in_=wk[:, 0])
        nc.sync.dma_start(out=xk_sb[:, 0], in_=xk[:, 0])
        nc.sync.dma_start(out=xk_sb[:, 1], in_=xk[:, 1])
        nc.sync.dma_start(out=wq_sb[:, 0], in_=wq[:, 0])
        nc.sync.dma_start(out=bq_sb, in_=bq)
        nc.sync.dma_start(out=xq_sb[:, 0], in_=xq[:, 0])
        nc.sync.dma_start(out=xq_sb[:, 1], in_=xq[:, 1])
        for hc in range(2, NHC):
            nc.sync.dma_start(out=xk_sb[:, hc], in_=xk[:, hc])
        nc.sync.dma_start(out=wv_sb[:, 0], in_=wv[:, 0])
        for hc in range(NHC):
            nc.sync.dma_start(out=xv_sb[:, hc], in_=xv[:, hc])
        nc.sync.dma_start(out=idm_sb, in_=idm)
        for hc in range(2, NHC):
            nc.sync.dma_start(out=xq_sb[:, hc], in_=xq[:, hc])
        nc.sync.dma_start(out=wk_sb[:, 1:NPAIR], in_=wk[:, 1:NPAIR])
        nc.sync.dma_start(out=wq_sb[:, 1:NPAIR], in_=wq[:, 1:NPAIR])
        nc.sync.dma_start(out=wv_sb[:, 1:NPAIR], in_=wv[:, 1:NPAIR])
        nc.sync.dma_start(out=wo_sb, in_=wo)

        def make_pair_tiles(hp):
            qT = ring.tile([128, S], BF16, tag="qT", name=f"qT{hp}")
            kT = ring.tile([128, S], BF16, tag="kT", name=f"kT{hp}")
            vaug = ring.tile([128, NTT, 2, D + 1], BF16, tag="vaug",
                             name=f"vaug{hp}")
            aoT = ring.tile([128, S], BF16, tag="aoT", name=f"aoT{hp}")
            nc.vector.memset(vaug[:, :, :, D:D + 1], 1.0)
            return dict(qT=qT, kT=kT, vaug=vaug, aoT=aoT)

        def kqchunk(hp, t, ck, half, which):
            """Half-width (256-token) k or q projection chunk: small enough
            (~0.9us PE) to interleave without starving the Act engine."""
            hc = ck * 2 + half
            t0 = hc * 256
            x_sb, w_sb = (xk_sb, wk_sb) if which == "k" else (xq_sb, wq_sb)
            ps = pp.tile([128, 256], F32, tag="pp", name="pskq",
                         padded_shape=[128, CKW])
            for k in range(KT):
                nc.tensor.matmul(
                    ps,
                    (w_sb[:, hp, k, :]),
                    (x_sb[:, hc, k, :]),
                    start=(k == 0),
                    stop=(k == KT - 1),
                )
            if which == "k":
                nc.vector.tensor_copy(out=t["kT"][:, t0:t0 + 256], in_=ps)
            else:
                nc.vector.tensor_scalar(
                    out=t["qT"][:, t0:t0 + 256],
                    in0=ps,
                    scalar1=bq_sb[:, hp:hp + 1],
                    scalar2=None,
                    op0=ALU.add,
                )

        def vchunk(hp, t, tt):
            hc, j = tt // 2, tt % 2
            ps = pp.tile([128, 128], F32, tag="pp", name="psv",
                         padded_shape=[128, CKW])
            for k in range(KT):
                nc.tensor.matmul(
                    ps,
                    (xv_sb[:, hc, k, j * 128:(j + 1) * 128]),
                    (wv_sb[:, hp, k, :]),
                    start=(k == 0),
                    stop=(k == KT - 1),
                )
            nc.vector.tensor_copy(
                out=t["vaug"][:, tt, :, 0:D],
                in_=ps.rearrange("p (h d) -> p h d", d=D),
            )

        # queue items are (est_PE_ns, deadline_iter_or_None, closure).
        # DEADLINES ARE A CORRECTNESS CONTRACT: the Tile framework registers
        # dependencies in emission order, so a producer (e.g. a v-projection
        # copy) must be EMITTED before any instruction that reads it; the
        # deadline forces the pop by the given attention iteration.
        def proj_ops(hp, t):
            """Pair hp's projections as (main, late): main = all of k and v
            plus q chunk 0, which must be emitted before pair hp's attention
            iterations that read them (they ride the PREVIOUS pair's fill
            queue, with a drain-by-end deadline); late = q chunks 1-3,
            needed only at their own query blocks (they ride pair hp's OWN
            queue, which balances PE load across pairs)."""
            main, late = [], []
            for ck in range(NCK):
                for half in range(2):
                    main.append((900, 56, lambda ck=ck, half=half:
                                 kqchunk(hp, t, ck, half, "k")))
            for half in range(2):
                main.append((900, 56, lambda half=half:
                             kqchunk(hp, t, 0, half, "q")))
            for tt in range(NTT):
                main.append((480, 56, lambda tt=tt: vchunk(hp, t, tt)))
            for ck in range(1, NCK):
                for half in range(2):
                    late.append((900, 16 * ck - 2, lambda ck=ck, half=half:
                                 kqchunk(hp, t, ck, half, "q")))
            return main, late

        def phasec_tile(hp, aoT_pair, ttile, eb):
            """Project one [128, 512] partial-out tile through Wo for a PAIR
            of head-pairs (hp-1, hp): contracting both in one PSUM group
            halves the copy and DMA traffic. Runs for hp in {1, 3}."""
            ps = pp.tile([128, 512], F32, tag="pp", name="psc")
            for i, hpc in enumerate((hp - 1, hp)):
                nc.tensor.matmul(
                    ps,
                    (aoT_pair[i][:, ttile * 128:(ttile + 1) * 128]),
                    (wo_sb[:, hpc, eb * 512:(eb + 1) * 512]),
                    start=(i == 0),
                    stop=(i == 1),
                )
            ot = otpool.tile([128, 512], F32, tag="ot")
            nc.vector.tensor_copy(out=ot, in_=ps)
            nc.sync.dma_start(
                out=out[hp // 2, ttile * 128:(ttile + 1) * 128,
                        eb * 512:(eb + 1) * 512],
                in_=ot,
            )

        def sc_exp(t, sb, h_in, tt2):
            """Scores for two token tiles + their exp, returning the bf16
            exp tile (the attnv stationary operand)."""
            p0 = h_in * 64
            sc = scp.tile([128, 2, CKW], F32, tag="sc", name="sc")
            for j in range(2):
                tt = tt2 * 2 + j
                nc.tensor.matmul(
                    sc[:, j, :],
                    (t["kT"][p0:p0 + 64, tt * 128:(tt + 1) * 128]),
                    (t["qT"][p0:p0 + 64, sb * CKW:(sb + 1) * CKW]),
                    start=True,
                    stop=True,
                )
            ex = expool.tile([128, 2, CKW], BF16, tag="ex", name="ex")
            nc.scalar.activation(out=ex, in_=sc, func=AF.Exp)
            return ex

        from collections import deque

        def drain(queue, credit, it, rate):
            """Emit deadline-due items (anywhere in the queue, order kept),
            then budget-paced head items."""
            if any(d is not None and d <= it for _, d, _ in queue):
                keep = deque()
                while queue:
                    cost, dl, f = queue.popleft()
                    if dl is not None and dl <= it:
                        credit[0] = max(credit[0] - cost, -3000)
                        f()
                    else:
                        keep.append((cost, dl, f))
                queue.extend(keep)
            credit[0] = min(credit[0] + rate, 3000)
            while queue and credit[0] >= queue[0][0]:
                cost, _, f = queue.popleft()
                credit[0] -= cost
                f()

        def attention(hp, t, fill, defer_tail, next_t=None, pre=()):
            """Pair hp's attention. A deque of independent PE work (next
            pair's projections, finished query blocks' transposes + Wo
            tiles) is drained between iterations so the PE keeps feeding
            Act; whatever is left at the end is returned (run inside the
            next pair's attention) when defer_tail. The last iterations
            emit the NEXT pair's first scores+exps (returned as `pre`) so
            the Act engine never idles across the pair boundary."""
            qT, kT, vaug, aoT = t["qT"], t["kT"], t["vaug"], t["aoT"]
            queue = deque(fill)
            credit = [2600]
            pre = list(pre)
            next_pre = []
            for sb in range(NSB):
                o_sb = npool.tile([128, 4, 128], BF16, tag="osb", bufs=3,
                                  name=f"osb{hp}_{sb}")
                for h_in in range(2):
                    p0 = h_in * 64
                    o_ps = opp.tile([128, 4, D + 1], F32, tag="opp",
                                    name=f"ops{hp}_{sb}_{h_in}")
                    exs = []
                    for tt2 in range(8):
                        it = (sb * 2 + h_in) * 8 + tt2
                        if pre:
                            exs.append(pre.pop(0))
                        else:
                            exs.append(sc_exp(t, sb, h_in, tt2))
                        if (next_t is not None and sb == NSB - 1
                                and h_in == 1 and tt2 >= 6):
                            next_pre.append(
                                sc_exp(next_t, 0, 0, tt2 - 6))
                        # fill the exp-wait gap with independent matmuls,
                        # paced by estimated PE-time so Act never starves;
                        # deadline-due items are emitted unconditionally
                        drain(queue, credit, it,
                              700 if hp == NPAIR - 1 else 500)

                    # attn@v: PSUM accumulation groups must not interleave
                    # within a bank, so each query tile's accumulation is a
                    # contiguous run of matmuls; runs go through the queue
                    # (deadlines keep them inside the next ~12 iterations)
                    # in half-unit chunks so the PE exec window stays small.
                    def avhalf(st, lo, exs=exs, o_ps=o_ps, h_in=h_in):
                        for tt in range(lo, lo + 8):
                            nc.tensor.matmul(
                                o_ps[:, st, :],
                                (exs[tt // 2][:, tt % 2,
                                              st * 128:(st + 1) * 128]),
                                (vaug[:, tt, h_in, :]),
                                start=(tt == 0),
                                stop=(tt == NTT - 1),
                            )

                    def norms(o_ps=o_ps, o_sb=o_sb, p0=p0):
                        for st in range(4):
                            rc = npool.tile([128, 1], F32, tag="rc", bufs=8,
                                            name="rc")
                            nc.vector.reciprocal(out=rc,
                                                 in_=o_ps[:, st, D:D + 1])
                            nc.vector.tensor_scalar(
                                out=o_sb[:, st, p0:p0 + 64],
                                in0=o_ps[:, st, 0:D],
                                scalar1=rc,
                                scalar2=None,
                                op0=ALU.mult,
                            )

                    if hp == 0 and sb == 0:
                        # warmup: xv is still streaming in; keep these out
                        # of the PE window until it lands
                        ubase, nbase = 16 + h_in * 8, 24 + h_in * 8
                    else:
                        ubase = (sb * 2 + h_in) * 8 + 8
                        nbase = ubase + 9
                    for st in range(4):
                        for half in range(2):
                            queue.append(
                                (240, ubase + 2 * st + half,
                                 lambda st=st, half=half, f=avhalf:
                                     f(st, half * 8))
                            )
                    queue.append((260, nbase, lambda f=norms: f()))

                def transp(st, sb=sb, o_sb=o_sb):
                    tp = pp.tile([128, 128], BF16, tag="pp", name="tp")
                    nc.tensor.transpose(tp, o_sb[:, st, :], idm_sb)
                    nc.vector.tensor_copy(
                        out=aoT[:, sb * CKW + st * 128:sb * CKW + (st + 1) * 128],
                        in_=tp,
                    )

                # this query block's transposes (+ output projection on odd
                # pairs): FIFO order after the norms above guarantees their
                # o_sb/aoT inputs are emitted first; the deadlines bound how
                # long they linger (tile-ring reuse in later pairs assumes
                # every read is emitted within its own pair)
                for st in range(4):
                    queue.append((160, sb * 16 + 40,
                                  lambda st=st, f=transp: f(st)))
                    if hp % 2 == 1:
                        ao2 = (tiles[hp - 1]["aoT"], aoT)
                        for eb in range(2):
                            queue.append(
                                (500, sb * 16 + 42,
                                 lambda st=st, eb=eb, sb=sb, ao2=ao2:
                                    phasec_tile(hp, ao2, sb * 4 + st, eb))
                            )
            # emission order is the dependency contract: everything must be
            # emitted inside its own pair, before the next pairs reuse the
            # vaug/o_ps/o_sb/aoT ring slots
            while queue:
                queue.popleft()[2]()
            return [], next_pre

        # ---- software pipeline over the 4 head pairs ----
        # pair 0: k chunk 0 + q chunk 0 inline (they gate the first scores);
        # everything else goes through the fill queue like later pairs'.
        # Its own k/v chunks carry per-iteration deadlines: scores at
        # iteration 2*ck read k chunk ck, attnv at iteration tt//2+1 reads
        # v chunk tt -- the deadline guarantees the producer is EMITTED
        # first (the Tile framework's dependencies follow emission order).
        tiles = {0: make_pair_tiles(0)}
        main0, late0 = proj_ops(0, tiles[0])
        inline0 = main0[0:2] + main0[8:10]  # k chunk-0 halves, q chunk-0
        for _, _, f in inline0:
            f()
        rest0 = (
            [(c, 2 * (1 + i // 2) - 1, f)
             for i, (c, _, f) in enumerate(main0[2:8])]      # k chunks 1-3
            + [(c, 10 + tt // 4, f)
               for tt, (c, _, f) in enumerate(main0[10:])]   # v chunks
        )
        tail = rest0 + late0
        late = []
        pre = []
        for hp in range(NPAIR):
            fill = list(late) + list(tail)
            late = []
            next_t = None
            if hp + 1 < NPAIR:
                tiles[hp + 1] = make_pair_tiles(hp + 1)
                main_n, late = proj_ops(hp + 1, tiles[hp + 1])
                fill += main_n
                next_t = tiles[hp + 1]
            tail, pre = attention(hp, tiles[hp], fill,
                                  defer_tail=hp + 1 < NPAIR,
                                  next_t=next_t, pre=pre)

    nc.compile()
    return nc


_PROG = {}


def _get_prog(S=2048, num_devices=8):
    key = (S, num_devices)
    if key not in _PROG:
        _PROG[key] = build_program(S, num_devices)
    return _PROG[key]


def _tile_x(x2d):
    # [S, E] slice -> [128, S//256, KT, 256] bf16 (half-chunk-major) with
    # element (p, hc, k, t) = x2d[hc*256 + t, k*128 + p]
    S = x2d.shape[0]
    xt = np.ascontiguousarray(x2d.T.astype(np.float32))   # [E, S]
    return np.ascontiguousarray(
        xt.reshape(KT, 128, S // 256, 256).transpose(1, 2, 0, 3)
    ).astype(BF_NP)


def _tile_w(weff, ch0):
    # [E, CH] slice -> [128, NPAIR, KT, 128] bf16 (pair-major) with
    # element (p, hp, k, c) = weff[k*128 + p, ch0 + hp*128 + c]
    w = weff[:, ch0:ch0 + CH]
    return np.ascontiguousarray(
        w.reshape(KT, 128, NPAIR, 128).transpose(1, 2, 0, 3).astype(np.float32)
    ).astype(BF_NP)


def prep_in_maps(x_q, x_k, x_v, Wq, bq, Aq, Bq, Wk, bk, Wv, bv, Av, Bv, Wo, bo):
    x_q = np.asarray(x_q, np.float32)
    x_k = np.asarray(x_k, np.float32)
    x_v = np.asarray(x_v, np.float32)
    scaling = 2.0  # lora_alpha / r = 32 / 16
    wq_eff = (np.asarray(Wq).T + (np.asarray(Aq) @ np.asarray(Bq)) * scaling).astype(
        np.float32
    )
    wv_eff = (np.asarray(Wv).T + (np.asarray(Av) @ np.asarray(Bv)) * scaling).astype(
        np.float32
    )
    wk_s = (np.asarray(Wk).T / 8.0).astype(np.float32)  # sqrt(D) folded in
    # bk is dropped: q.(k+bk) shifts every score of a query equally, which
    # softmax cancels exactly.
    bq = np.asarray(bq, np.float32)
    woT = np.ascontiguousarray(np.asarray(Wo).T.astype(np.float32))
    idm = np.eye(128, dtype=np.float32).astype(BF_NP)

    nbatch = x_q.shape[1]
    in_maps = []
    for c in range(2 * nbatch):
        b = c // 2
        hg = c % 2
        ch0 = hg * CH
        wo_c = np.ascontiguousarray(
            woT[ch0:ch0 + CH, :].reshape(NPAIR, 128, E).transpose(1, 0, 2)
        ).astype(BF_NP)
        in_maps.append({
            "xq": _tile_x(x_q[:, b, :]),
            "xk": _tile_x(x_k[:, b, :]),
            "xv": _tile_x(x_v[:, b, :]),
            "wq": _tile_w(wq_eff, ch0),
            "wk": _tile_w(wk_s, ch0),
            "wv": _tile_w(wv_eff, ch0),
            "wo": wo_c,
            "bq": np.ascontiguousarray(
                bq[ch0:ch0 + CH].reshape(NPAIR, 128).T
            ),
            "idm": idm,
        })
    return in_maps


def gather_out(results, nbatch, bias_row):
    # out[s, b, :] = sum over the two cores' two half-partials + bias_row
    outs = []
    for b in range(nbatch):
        acc = results[2 * b]["out"].sum(axis=0)
        acc = acc + results[2 * b + 1]["out"].sum(axis=0)
        outs.append(acc + bias_row)
    return np.stack(outs, axis=1).astype(np.float32)


def kernel(**inputs):
    nc = _get_prog(2048, 8)
    in_maps = prep_in_maps(**inputs)
    # bv contributes bv @ Wo.T (attention weights sum to 1); bo is the
    # plain output bias. Both are per-row constants added on the host.
    bias_row = (
        np.asarray(inputs["bv"], np.float64) @ np.asarray(inputs["Wo"], np.float64).T
        + np.asarray(inputs["bo"], np.float64)
    ).astype(np.float32)
    res = run_bass_kernel_spmd(nc, in_maps, core_ids=list(range(NCORES)))
    return gather_out(res.results, B, bias_row)


# revision 85
# speedup vs baseline: 1.4501x; 1.1010x over previous
"""LoRA multi-head attention kernel for 8 Trainium2 NeuronCores.

Problem: q = x_q@(Wq.T + Aq@Bq*2) + bq ; k = x_k@Wk.T + bk ;
         v = x_v@(Wv.T + Av@Bv*2) + bv ; MHA over 16 heads, D=64,
         out = attn_out @ Wo.T + bo.   Shapes: x [2048, 4, 1024].

Sharding: core c handles batch b = c//2 and head-group hg = c%2
(8 heads = 512 output channels), processed as 4 head-PAIRS so every
projection matmul uses the full 128 PE output partitions.

Math simplifications (exact):
  * LoRA weights merged on host; 1/sqrt(D) folded into Wk.
  * bk dropped entirely: q.(k+bk) adds a per-query constant to every
    score, which softmax cancels.
  * bv dropped on-device: sum(attn)=1, so + bv@Wo.T is a constant
    row added on the host. bo is also added on the host.
  * Per-pair partial output projections are summed on the host
    (together with the partner core's partials).

Per pair the attention runs "orientation B": scores land as
sc[t, s] tiles, exp'd by the Act engine into bf16, and attn@v uses
ex[t, s-tile] as the *stationary* operand with [v | 1] as the moving
operand (free dim 65).  That halves the tensor-engine rows of attn@v
versus the [d+1, s]-output orientation, and the softmax denominator
(column 64 of the output) becomes a per-partition scalar, so
normalization is a cheap DVE tensor_scalar.  The normalized [s, c]
output is PE-transposed to [c, s] to feed the Wo projection.

The emission interleaves the NEXT pair's projections and the PREVIOUS
pair's Wo-projection tiles between attention iterations so the tensor
engine never idles while the Act engine grinds the exps (the Act
engine's 262k exp-elements/core at 0.83 ns/elem is nearly as expensive
as all matmuls).

Dtypes: x/W inputs bf16 (halves DMA), qT/kT kept f32r, exp output and
the v/Wo path bf16.
"""

import sys

import numpy as np

sys.path.insert(0, "/opt/trn_rl_repo")

from contextlib import ExitStack  # noqa: E402

import ml_dtypes  # noqa: E402

import concourse.bass as bass  # noqa: E402
import concourse.tile as tile  # noqa: E402
from concourse import bacc, mybir  # noqa: E402
from concourse.bass_utils import run_bass_kernel_spmd  # noqa: E402

F32 = mybir.dt.float32
F32R = mybir.dt.float32r
BF16 = mybir.dt.bfloat16
AF = mybir.ActivationFunctionType
ALU = mybir.AluOpType

E = 1024
D = 64
CH = 512           # output channels per core
NPAIR = 4          # head pairs per core
KT = E // 128      # 8 k-tiles over the E contraction
NCORES = 8
B = 4
BF_NP = ml_dtypes.bfloat16


def build_program(S=2048, num_devices=8):
    NTT = S // 128          # 16 token tiles
    NSB = S // 512          # 4 query blocks
    NCK = S // 512          # 4 projection token chunks
    CKW = 512

    nc = bacc.Bacc(
        "TRN2", target_bir_lowering=False, debug=False, num_devices=num_devices
    )

    def dram(name, shape, dt=F32, out=False):
        kind = "ExternalOutput" if out else "ExternalInput"
        return nc.dram_tensor(name, shape, dt, kind=kind).ap()

    NHC = S // 256          # 8 half-chunks per x tensor
    # x: half-chunk-major so each DMA writes (and each reader reads) one
    # contiguous block -- the tile framework's range-based subtile deps
    # would otherwise serialize readers on ALL of a tensor's chunk DMAs
    xq = dram("xq", [128, NHC, KT, 256], BF16)
    xk = dram("xk", [128, NHC, KT, 256], BF16)
    xv = dram("xv", [128, NHC, KT, 256], BF16)
    # weights: pair-major for the same reason
    wq = dram("wq", [128, NPAIR, KT, 128], BF16)
    wk = dram("wk", [128, NPAIR, KT, 128], BF16)
    wv = dram("wv", [128, NPAIR, KT, 128], BF16)
    wo = dram("wo", [128, NPAIR, E], BF16)
    bq = dram("bq", [128, NPAIR])
    idm = dram("idm", [128, 128], BF16)
    out = dram("out", [NPAIR // 2, S, E], out=True)

    with tile.TileContext(nc) as tc, ExitStack() as top:
        persist = top.enter_context(tc.tile_pool(name="persist", bufs=1))
        ring = top.enter_context(tc.tile_pool(name="ring", bufs=2))
        expool = top.enter_context(tc.tile_pool(name="expool", bufs=14))
        npool = top.enter_context(tc.tile_pool(name="npool", bufs=8))
        otpool = top.enter_context(tc.tile_pool(name="otpool", bufs=4))
        pp = top.enter_context(tc.tile_pool(name="pp", bufs=2, space="PSUM"))
        scp = top.enter_context(tc.tile_pool(name="scp", bufs=2, space="PSUM"))
        opp = top.enter_context(tc.tile_pool(name="opp", bufs=2, space="PSUM"))

        xq_sb = persist.tile([128, NHC, KT, 256], BF16)
        xk_sb = persist.tile([128, NHC, KT, 256], BF16)
        xv_sb = persist.tile([128, NHC, KT, 256], BF16)
        wq_sb = persist.tile([128, NPAIR, KT, 128], BF16)
        wk_sb = persist.tile([128, NPAIR, KT, 128], BF16)
        wv_sb = persist.tile([128, NPAIR, KT, 128], BF16)
        wo_sb = persist.tile([128, NPAIR, E], BF16)
        bq_sb = persist.tile([128, NPAIR], F32)
        idm_sb = persist.tile([128, 128], BF16)

        # DMA order tuned for startup: the first scores only need pair-0's
        # wk/wq column slices, a half xk chunk and the first xq chunk
        # (~1.8 MB); everything else streams in behind while attention is
        # already running (the sim's engines let ready instructions bypass
        # ones parked on late DMAs).
        nc.sync.dma_start(out=wk_sb[:, 0], in_=wk[:, 0])
        nc.sync.dma_start(out=xk_sb[:, 0], in_=xk[:, 0])
        nc.sync.dma_start(out=xk_sb[:, 1], in_=xk[:, 1])
        nc.sync.dma_start(out=wq_sb[:, 0], in_=wq[:, 0])
        nc.sync.dma_start(out=bq_sb, in_=bq)
        nc.sync.dma_start(out=xq_sb[:, 0], in_=xq[:, 0])
        nc.sync.dma_start(out=xq_sb[:, 1], in_=xq[:, 1])
        for hc in range(2, NHC):
            nc.sync.dma_start(out=xk_sb[:, hc], in_=xk[:, hc])
        nc.sync.dma_start(out=wv_sb[:, 0], in_=wv[:, 0])
        for hc in range(NHC):
            nc.sync.dma_start(out=xv_sb[:, hc], in_=xv[:, hc])
        nc.sync.dma_start(out=idm_sb, in_=idm)
        for hc in range(2, NHC):
            nc.sync.dma_start(out=xq_sb[:, hc], in_=xq[:, hc])
        nc.sync.dma_start(out=wk_sb[:, 1:NPAIR], in_=wk[:, 1:NPAIR])
        nc.sync.dma_start(out=wq_sb[:, 1:NPAIR], in_=wq[:, 1:NPAIR])
        nc.sync.dma_start(out=wv_sb[:, 1:NPAIR], in_=wv[:, 1:NPAIR])
        nc.sync.dma_start(out=wo_sb, in_=wo)

        def make_pair_tiles(hp):
            # NOTE: the vaug ones-column memset is NOT emitted here -- it
            # rides the fill queue so it lands after the carried reads of
            # the ring slot it overwrites
            qT = ring.tile([128, S], BF16, tag="qT", name=f"qT{hp}")
            kT = ring.tile([128, S], BF16, tag="kT", name=f"kT{hp}")
            vaug = ring.tile([128, NTT, 2, D + 1], BF16, tag="vaug",
                             name=f"vaug{hp}")
            aoT = ring.tile([128, S], BF16, tag="aoT", name=f"aoT{hp}")
            return dict(qT=qT, kT=kT, vaug=vaug, aoT=aoT)

        def kqchunk(hp, t, ck, half, which):
            """Half-width (256-token) k or q projection chunk: small enough
            (~0.9us PE) to interleave without starving the Act engine."""
            hc = ck * 2 + half
            t0 = hc * 256
            x_sb, w_sb = (xk_sb, wk_sb) if which == "k" else (xq_sb, wq_sb)
            ps = pp.tile([128, 256], F32, tag="pp", name="pskq",
                         padded_shape=[128, CKW])
            for k in range(KT):
                nc.tensor.matmul(
                    ps,
                    (w_sb[:, hp, k, :]),
                    (x_sb[:, hc, k, :]),
                    start=(k == 0),
                    stop=(k == KT - 1),
                )
            if which == "k":
                nc.vector.tensor_copy(out=t["kT"][:, t0:t0 + 256], in_=ps)
            else:
                nc.vector.tensor_scalar(
                    out=t["qT"][:, t0:t0 + 256],
                    in0=ps,
                    scalar1=bq_sb[:, hp:hp + 1],
                    scalar2=None,
                    op0=ALU.add,
                )

        def vchunk(hp, t, tt):
            hc, j = tt // 2, tt % 2
            ps = pp.tile([128, 128], F32, tag="pp", name="psv",
                         padded_shape=[128, CKW])
            for k in range(KT):
                nc.tensor.matmul(
                    ps,
                    (xv_sb[:, hc, k, j * 128:(j + 1) * 128]),
                    (wv_sb[:, hp, k, :]),
                    start=(k == 0),
                    stop=(k == KT - 1),
                )
            nc.vector.tensor_copy(
                out=t["vaug"][:, tt, :, 0:D],
                in_=ps.rearrange("p (h d) -> p h d", d=D),
            )

        # queue items are (est_PE_ns, deadline_iter_or_None, closure).
        # DEADLINES ARE A CORRECTNESS CONTRACT: the Tile framework registers
        # dependencies in emission order, so a producer (e.g. a v-projection
        # copy) must be EMITTED before any instruction that reads it; the
        # deadline forces the pop by the given attention iteration.
        def proj_ops(hp, t):
            """Pair hp's projections as (main, late): main = all of k and v
            plus q chunk 0, which must be emitted before pair hp's attention
            iterations that read them (they ride the PREVIOUS pair's fill
            queue, with a drain-by-end deadline); late = q chunks 1-3,
            needed only at their own query blocks (they ride pair hp's OWN
            queue, which balances PE load across pairs)."""
            main, late = [], []
            for ck in range(NCK):
                for half in range(2):
                    main.append((900, 56, lambda ck=ck, half=half:
                                 kqchunk(hp, t, ck, half, "k")))
            for half in range(2):
                main.append((900, 56, lambda half=half:
                             kqchunk(hp, t, 0, half, "q")))
            for tt in range(NTT):
                main.append((480, 56, lambda tt=tt: vchunk(hp, t, tt)))
            for ck in range(1, NCK):
                for half in range(2):
                    late.append((900, 16 * ck - 2, lambda ck=ck, half=half:
                                 kqchunk(hp, t, ck, half, "q")))
            return main, late

        def phasec_tile(hp, aoT_pair, ttile, eb, act_copy=False):
            """Project one [128, 512] partial-out tile through Wo for a PAIR
            of head-pairs (hp-1, hp): contracting both in one PSUM group
            halves the copy and DMA traffic. Runs for hp in {1, 3}."""
            ps = pp.tile([128, 512], F32, tag="pp", name="psc")
            for i, hpc in enumerate((hp - 1, hp)):
                nc.tensor.matmul(
                    ps,
                    (aoT_pair[i][:, ttile * 128:(ttile + 1) * 128]),
                    (wo_sb[:, hpc, eb * 512:(eb + 1) * 512]),
                    start=(i == 0),
                    stop=(i == 1),
                )
            ot = otpool.tile([128, 512], F32, tag="ot")
            if act_copy:
                nc.scalar.copy(out=ot, in_=ps)
            else:
                nc.vector.tensor_copy(out=ot, in_=ps)
            nc.sync.dma_start(
                out=out[hp // 2, ttile * 128:(ttile + 1) * 128,
                        eb * 512:(eb + 1) * 512],
                in_=ot,
            )

        def sc_exp(t, sb, h_in, tt2):
            """Scores for two token tiles + their exp, returning the bf16
            exp tile (the attnv stationary operand)."""
            p0 = h_in * 64
            sc = scp.tile([128, 2, CKW], F32, tag="sc", name="sc")
            for j in range(2):
                tt = tt2 * 2 + j
                nc.tensor.matmul(
                    sc[:, j, :],
                    (t["kT"][p0:p0 + 64, tt * 128:(tt + 1) * 128]),
                    (t["qT"][p0:p0 + 64, sb * CKW:(sb + 1) * CKW]),
                    start=True,
                    stop=True,
                )
            ex = expool.tile([128, 2, CKW], BF16, tag="ex", name="ex")
            nc.scalar.activation(out=ex, in_=sc, func=AF.Exp)
            return ex

        from collections import deque

        def drain(queue, credit, it, rate):
            """Emit deadline-due items (anywhere in the queue, order kept),
            then budget-paced head items."""
            if any(d is not None and d <= it for _, d, _ in queue):
                keep = deque()
                while queue:
                    cost, dl, f = queue.popleft()
                    if dl is not None and dl <= it:
                        credit[0] = max(credit[0] - cost, -3000)
                        f()
                    else:
                        keep.append((cost, dl, f))
                queue.extend(keep)
            credit[0] = min(credit[0] + rate, 3000)
            while queue and credit[0] >= queue[0][0]:
                cost, _, f = queue.popleft()
                credit[0] -= cost
                f()

        def attention(hp, t, fill, defer_tail, next_t=None, pre=()):
            """Pair hp's attention. A deque of independent PE work (next
            pair's projections, finished query blocks' transposes + Wo
            tiles) is drained between iterations so the PE keeps feeding
            Act; whatever is left at the end is returned (run inside the
            next pair's attention) when defer_tail. The last iterations
            emit the NEXT pair's first scores+exps (returned as `pre`) so
            the Act engine never idles across the pair boundary."""
            qT, kT, vaug, aoT = t["qT"], t["kT"], t["vaug"], t["aoT"]
            queue = deque(fill)
            credit = [2600]
            pre = list(pre)
            next_pre = []
            for sb in range(NSB):
                o_sb = npool.tile([128, 4, 128], BF16, tag="osb", bufs=3,
                                  name=f"osb{hp}_{sb}")
                for h_in in range(2):
                    p0 = h_in * 64
                    o_ps = opp.tile([128, 4, D + 1], F32, tag="opp",
                                    name=f"ops{hp}_{sb}_{h_in}")
                    exs = []
                    for tt2 in range(8):
                        it = (sb * 2 + h_in) * 8 + tt2
                        if pre:
                            exs.append(pre.pop(0))
                        else:
                            exs.append(sc_exp(t, sb, h_in, tt2))
                        if (next_t is not None and sb == NSB - 1
                                and h_in == 1 and tt2 >= 4):
                            next_pre.append(
                                sc_exp(next_t, 0, 0, tt2 - 4))
                        # fill the exp-wait gap with independent matmuls,
                        # paced by estimated PE-time so Act never starves;
                        # deadline-due items are emitted unconditionally
                        drain(queue, credit, it,
                              700 if hp == NPAIR - 1 else 500)

                    # attn@v: PSUM accumulation groups must not interleave
                    # within a bank, so each query tile's accumulation is a
                    # contiguous run of matmuls; runs go through the queue
                    # (deadlines keep them inside the next ~12 iterations)
                    # in half-unit chunks so the PE exec window stays small.
                    def avhalf(st, lo, exs=exs, o_ps=o_ps, h_in=h_in):
                        for tt in range(lo, lo + 8):
                            nc.tensor.matmul(
                                o_ps[:, st, :],
                                (exs[tt // 2][:, tt % 2,
                                              st * 128:(st + 1) * 128]),
                                (vaug[:, tt, h_in, :]),
                                start=(tt == 0),
                                stop=(tt == NTT - 1),
                            )

                    def norms(o_ps=o_ps, o_sb=o_sb, p0=p0):
                        for st in range(4):
                            rc = npool.tile([128, 1], F32, tag="rc", bufs=8,
                                            name="rc")
                            nc.vector.reciprocal(out=rc,
                                                 in_=o_ps[:, st, D:D + 1])
                            nc.vector.tensor_scalar(
                                out=o_sb[:, st, p0:p0 + 64],
                                in0=o_ps[:, st, 0:D],
                                scalar1=rc,
                                scalar2=None,
                                op0=ALU.mult,
                            )

                    if hp == 0 and sb == 0:
                        # warmup: xv is still streaming in; keep these out
                        # of the PE window until it lands
                        ubase, nbase = 16 + h_in * 8, 24 + h_in * 8
                    else:
                        ubase = (sb * 2 + h_in) * 8 + 8
                        nbase = ubase + 9
                    for st in range(4):
                        for half in range(2):
                            queue.append(
                                (240, ubase + 2 * st + half,
                                 lambda st=st, half=half, f=avhalf:
                                     f(st, half * 8))
                            )
                    queue.append((260, nbase, lambda f=norms: f()))

                def transp(st, sb=sb, o_sb=o_sb):
                    tp = pp.tile([128, 128], BF16, tag="pp", name="tp")
                    nc.tensor.transpose(tp, o_sb[:, st, :], idm_sb)
                    nc.vector.tensor_copy(
                        out=aoT[:, sb * CKW + st * 128:sb * CKW + (st + 1) * 128],
                        in_=tp,
                    )

                # this query block's transposes (+ output projection on odd
                # pairs): FIFO order after the norms above guarantees their
                # o_sb/aoT inputs are emitted first; the deadlines bound how
                # long they linger (tile-ring reuse in later pairs assumes
                # every read is emitted within its own pair)
                for st in range(4):
                    queue.append((160, sb * 16 + 56,
                                  lambda st=st, f=transp: f(st)))
                    if hp % 2 == 1:
                        ao2 = (tiles[hp - 1]["aoT"], aoT)
                        for eb in range(2):
                            queue.append(
                                (500, sb * 16 + 58,
                                 lambda st=st, eb=eb, sb=sb, ao2=ao2:
                                    phasec_tile(hp, ao2, sb * 4 + st, eb,
                                                act_copy=(hp == NPAIR - 1
                                                          and sb == NSB - 1
                                                          and eb == 1)))
                            )
            # emission order is the dependency contract. Deadline-56 items
            # (next pair's projections) have all fired by now; what remains
            # is the last block's attnv/norm/transpose/Wo work. When a next
            # pair exists, carry it re-stamped to fire in that pair's first
            # 8 iterations -- still BEFORE any ring-slot reuse is emitted
            # (o_ps writers fire at iteration >= 8, norms/o_sb at >= 17,
            # transposes/aoT at >= 40, and the vaug memset rides the queue
            # behind these) -- which keeps the PE fed across the boundary.
            if defer_tail:
                return [(c, 8 if (d is None or d > 8) else d, f)
                        for c, d, f in queue], next_pre
            while queue:
                queue.popleft()[2]()
            return [], next_pre

        # ---- software pipeline over the 4 head pairs ----
        # pair 0: k chunk 0 + q chunk 0 inline (they gate the first scores);
        # everything else goes through the fill queue like later pairs'.
        # Its own k/v chunks carry per-iteration deadlines: scores at
        # iteration 2*ck read k chunk ck, attnv at iteration tt//2+1 reads
        # v chunk tt -- the deadline guarantees the producer is EMITTED
        # first (the Tile framework's dependencies follow emission order).
        tiles = {0: make_pair_tiles(0)}
        nc.vector.memset(tiles[0]["vaug"][:, :, :, D:D + 1], 1.0)
        main0, late0 = proj_ops(0, tiles[0])
        inline0 = main0[0:2] + main0[8:10]  # k chunk-0 halves, q chunk-0
        for _, _, f in inline0:
            f()
        rest0 = (
            [(c, 2 * (1 + i // 2) - 1, f)
             for i, (c, _, f) in enumerate(main0[2:8])]      # k chunks 1-3
            + [(c, 10 + tt // 4, f)
               for tt, (c, _, f) in enumerate(main0[10:])]   # v chunks
        )
        tail = rest0 + late0
        late = []
        pre = []
        for hp in range(NPAIR):
            fill = list(late) + list(tail)
            late = []
            next_t = None
            if hp + 1 < NPAIR:
                tiles[hp + 1] = make_pair_tiles(hp + 1)
                main_n, late = proj_ops(hp + 1, tiles[hp + 1])
                vg = tiles[hp + 1]["vaug"]
                fill.append((60, 12, lambda vg=vg:
                             nc.vector.memset(vg[:, :, :, D:D + 1], 1.0)))
                fill += main_n
                next_t = tiles[hp + 1]
            tail, pre = attention(hp, tiles[hp], fill,
                                  defer_tail=hp + 1 < NPAIR,
                                  next_t=next_t, pre=pre)

    nc.compile()
    return nc


_PROG = {}


def _get_prog(S=2048, num_devices=8):
    key = (S, num_devices)
    if key not in _PROG:
        _PROG[key] = build_program(S, num_devices)
    return _PROG[key]


def _tile_x(x2d):
    # [S, E] slice -> [128, S//256, KT, 256] bf16 (half-chunk-major) with
    # element (p, hc, k, t) = x2d[hc*256 + t, k*128 + p]
    S = x2d.shape[0]
    xt = np.ascontiguousarray(x2d.T.astype(np.float32))   # [E, S]
    return np.ascontiguousarray(
        xt.reshape(KT, 128, S // 256, 256).transpose(1, 2, 0, 3)
    ).astype(BF_NP)


def _tile_w(weff, ch0):
    # [E, CH] slice -> [128, NPAIR, KT, 128] bf16 (pair-major) with
    # element (p, hp, k, c) = weff[k*128 + p, ch0 + hp*128 + c]
    w = weff[:, ch0:ch0 + CH]
    return np.ascontiguousarray(
        w.reshape(KT, 128, NPAIR, 128).transpose(1, 2, 0, 3).astype(np.float32)
    ).astype(BF_NP)


def prep_in_maps(x_q, x_k, x_v, Wq, bq, Aq, Bq, Wk, bk, Wv, bv, Av, Bv, Wo, bo):
    x_q = np.asarray(x_q, np.float32)
    x_k = np.asarray(x_k, np.float32)
    x_v = np.asarray(x_v, np.float32)
    scaling = 2.0  # lora_alpha / r = 32 / 16
    wq_eff = (np.asarray(Wq).T + (np.asarray(Aq) @ np.asarray(Bq)) * scaling).astype(
        np.float32
    )
    wv_eff = (np.asarray(Wv).T + (np.asarray(Av) @ np.asarray(Bv)) * scaling).astype(
        np.float32
    )
    wk_s = (np.asarray(Wk).T / 8.0).astype(np.float32)  # sqrt(D) folded in
    # bk is dropped: q.(k+bk) shifts every score of a query equally, which
    # softmax cancels exactly.
    bq = np.asarray(bq, np.float32)
    woT = np.ascontiguousarray(np.asarray(Wo).T.astype(np.float32))
    idm = np.eye(128, dtype=np.float32).astype(BF_NP)

    nbatch = x_q.shape[1]
    in_maps = []
    for c in range(2 * nbatch):
        b = c // 2
        hg = c % 2
        ch0 = hg * CH
        wo_c = np.ascontiguousarray(
            woT[ch0:ch0 + CH, :].reshape(NPAIR, 128, E).transpose(1, 0, 2)
        ).astype(BF_NP)
        in_maps.append({
            "xq": _tile_x(x_q[:, b, :]),
            "xk": _tile_x(x_k[:, b, :]),
            "xv": _tile_x(x_v[:, b, :]),
            "wq": _tile_w(wq_eff, ch0),
            "wk": _tile_w(wk_s, ch0),
            "wv": _tile_w(wv_eff, ch0),
            "wo": wo_c,
            "bq": np.ascontiguousarray(
                bq[ch0:ch0 + CH].reshape(NPAIR, 128).T
            ),
            "idm": idm,
        })
    return in_maps


def gather_out(results, nbatch, bias_row):
    # out[s, b, :] = sum over the two cores' two half-partials + bias_row
    outs = []
    for b in range(nbatch):
        acc = results[2 * b]["out"].sum(axis=0)
        acc = acc + results[2 * b + 1]["out"].sum(axis=0)
        outs.append(acc + bias_row)
    return np.stack(outs, axis=1).astype(np.float32)


def kernel(**inputs):
    nc = _get_prog(2048, 8)
    in_maps = prep_in_maps(**inputs)
    # bv contributes bv @ Wo.T (attention weights sum to 1); bo is the
    # plain output bias. Both are per-row constants added on the host.
    bias_row = (
        np.asarray(inputs["bv"], np.float64) @ np.asarray(inputs["Wo"], np.float64).T
        + np.asarray(inputs["bo"], np.float64)
    ).astype(np.float32)
    res = run_bass_kernel_spmd(nc, in_maps, core_ids=list(range(NCORES)))
    return gather_out(res.results, B, bias_row)
